# revision 17
# baseline (speedup 1.0000x reference)
"""Trainium2 Bass kernel for AgnosticNonlinearInteractionBlock (GNN message passing).

Sharding: edges partitioned by receiver across 8 cores; each core computes full
output rows for its 1250-node slice. No collectives.

v3 design (vs baseline):
  - receiver nodes attr-sorted into static G=160 slot ranges -> the mid linear
    and skip_tp fuse into per-attr [2C->C] weights applied per static attr-run
    (eliminates the dense skip_tp phase entirely)
  - per-edge y0/y1i factors folded into on-device-built scaled one-hot scatter
    stationaries Sy0..Sy3 (= S1 * y); messages become plain tensor_tensor
    products (2x DVE mode), wt read directly from PSUM
  - S1 / sidx / yq resident in SBUF (loaded once per NEFF, not per iteration)
  - per-block tile counts data-driven (balanced within attr ranges)
  - d_table double-buffered across iterations so KITERS>1 pipelines
"""

import sys

sys.path.insert(0, "/opt/trn_rl_repo")

import numpy as np
import ml_dtypes

BF16 = ml_dtypes.bfloat16

# Problem constants
N, E = 10000, 160000
C, A, R, H = 128, 10, 8, 64
AVG_NEI = 16.0
INV_SQRT3 = 1.0 / np.sqrt(3.0)

NCORES = 8
NPC = N // NCORES          # 1250 local nodes per core
G_ATTR = 160               # static slots per attr value
BLOCKS = (G_ATTR * A + 127) // 128   # 13 receiver blocks of 128 slots
LNPAD = BLOCKS * 128       # 1664 local padded node slots
N_PAD = 10112              # 79*128 padded node count for the up table
NT_UP = N_PAD // 128
MLP_CH = 384               # mlp chunk = 3 tiles worth of columns (768 edges)

# static run table: block -> list of (attr, col0, col1); last run extended to 128
_RUNS = []
for _b in range(BLOCKS):
    lo, hi = _b * 128, _b * 128 + 128
    rs = []
    for _a in range(A):
        s0, s1 = max(lo, _a * G_ATTR), min(hi, (_a + 1) * G_ATTR)
        if s1 > s0:
            rs.append([_a, s0 - lo, s1 - lo])
    rs[-1][2] = 128
    _RUNS.append([tuple(r) for r in rs])

_LAYOUT = {}   # set by _prep_host: caps (tiles per block), TILES, E_CAP
_CACHE = {}


def _pack_core(deg_local, attr_local):
    """Assign local nodes to slots: attr a -> slots [a*G, (a+1)*G); within the
    blocks an attr range spans, place high-degree nodes into the least edge-
    loaded block. Returns perm (slot -> local node id or -1) and per-block
    edge loads."""
    perm = np.full(LNPAD, -1, np.int64)
    load = np.zeros(BLOCKS)
    for a in range(A):
        nodes = np.where(attr_local == a)[0]
        assert len(nodes) <= G_ATTR, f"attr {a}: {len(nodes)} > {G_ATTR}"
        lo, hi = a * G_ATTR, (a + 1) * G_ATTR
        segs = []   # (block, slot0, nslots)
        for k in range(lo // 128, (hi - 1) // 128 + 1):
            s0, s1 = max(lo, 128 * k), min(hi, 128 * (k + 1))
            segs.append([k, s0, s1 - s0])
        cnt = {k: 0 for k, _, _ in segs}
        cap = {k: n for k, _, n in segs}
        s0of = {k: s for k, s, _ in segs}
        for n in nodes[np.argsort(-deg_local[nodes])]:
            k = min((k for k in cap if cnt[k] < cap[k]), key=lambda k: load[k])
            perm[s0of[k] + cnt[k]] = n
            cnt[k] += 1
            load[k] += deg_local[n]
    return perm, load


def _prep_host(node_attrs, node_feats, edge_attrs, edge_feats, edge_index,
               W_up0, W_up1, W_mlp1, W_mlp2, W_mlp3, W_mlp4,
               W_lin0, W_lin1, W_skip0, W_skip1):
    send = np.asarray(edge_index[0]).astype(np.int64)
    recv = np.asarray(edge_index[1]).astype(np.int64)
    ef = np.asarray(edge_feats, np.float32)
    ea = np.asarray(edge_attrs, np.float32)
    attr = np.asarray(node_attrs).argmax(1)
    deg = np.bincount(recv, minlength=N)

    # ---- per-core node permutation + per-block tile caps (core-invariant) ----
    perms, loads = [], []
    for m in range(NCORES):
        lo = m * NPC
        p, ld = _pack_core(deg[lo:lo + NPC], attr[lo:lo + NPC])
        perms.append(p)
        loads.append(ld)
    caps = np.ceil(np.stack(loads).max(0) / 128.0).astype(int) + 0
    caps = np.maximum(caps, 1)
    TILES = int(caps.sum())
    E_CAP = TILES * 128
    _LAYOUT["caps"] = tuple(int(c) for c in caps)
    _LAYOUT["TILES"] = TILES
    _LAYOUT["E_CAP"] = E_CAP
    tile0 = np.concatenate([[0], np.cumsum(caps)])  # first tile of block b

    # ---- shared weights ----
    w1 = (np.asarray(W_mlp1, np.float32) / np.sqrt(R)).astype(BF16)
    w2 = (np.asarray(W_mlp2, np.float32) / np.sqrt(H)).astype(BF16)
    w3 = (np.asarray(W_mlp3, np.float32) / np.sqrt(H)).astype(BF16)
    # w4 column-reordered: [ws1 | wv2 | ws2*inv_sqrt3 | wv1]
    w4f = np.asarray(W_mlp4, np.float32) / np.sqrt(H)
    w4 = np.concatenate([w4f[:, 0:C], w4f[:, 3 * C:4 * C],
                         w4f[:, C:2 * C] * INV_SQRT3, w4f[:, 2 * C:3 * C]],
                        axis=1).astype(BF16)                       # [64, 512]
    wup = (np.stack([np.asarray(W_up0, np.float32), np.asarray(W_up1, np.float32)])
           / np.sqrt(C)).astype(BF16)                             # [2,128,128]

    # fused tail weights per attr: [FS1|FS2|FV1|FV2], scale folded
    norm = np.sqrt(2 * C) * AVG_NEI
    fan = np.sqrt(C * A)
    sc = 1.0 / (norm * fan)
    wl0 = np.asarray(W_lin0, np.float64)
    wl1 = np.asarray(W_lin1, np.float64)
    wsk0 = np.asarray(W_skip0, np.float64)
    wsk1 = np.asarray(W_skip1, np.float64)
    wtail = np.zeros((A, 4, 128, 128), np.float64)
    for a in range(A):
        wtail[a, 0] = wl0[:C] @ wsk0[:, a, :] * sc
        wtail[a, 1] = wl0[C:] @ wsk0[:, a, :] * sc
        wtail[a, 2] = wl1[:C] @ wsk1[:, a, :] * sc
        wtail[a, 3] = wl1[C:] @ wsk1[:, a, :] * sc
    wtail = wtail.astype(BF16)                                    # [A,4,128,128]

    # node_feats transposed planes for phase 1: [4,128,N_PAD] -> [512, N_PAD]
    nfT = np.zeros((4, 128, N_PAD), np.float32)
    nfT[0, :, :N] = np.asarray(node_feats, np.float32)[:, :C].T
    v = np.asarray(node_feats, np.float32)[:, C:].reshape(N, C, 3)
    for i in range(3):
        nfT[1 + i, :, :N] = v[:, :, i].T
    nfT = nfT.reshape(512, N_PAD).astype(BF16)

    ident = np.eye(128, dtype=BF16)

    in_maps = []
    for m in range(NCORES):
        lo = m * NPC
        perm = perms[m]
        # slot of each local node
        slot_of = np.full(NPC, -1, np.int64)
        live = perm >= 0
        slot_of[perm[live]] = np.nonzero(live)[0]

        # bucket edges into blocks by receiver slot
        mask = (recv >= lo) & (recv < lo + NPC)
        eidx = np.nonzero(mask)[0]
        rslot = slot_of[recv[eidx] - lo]
        blk = rslot // 128
        perm_e = np.full(E_CAP, -1, np.int64)
        rcol = np.zeros(E_CAP, np.int64)
        for b in range(BLOCKS):
            be = eidx[blk == b]
            cap = caps[b] * 128
            assert len(be) <= cap, f"core {m} block {b}: {len(be)} > {cap}"
            o = tile0[b] * 128
            perm_e[o:o + len(be)] = be
            rcol[o:o + len(be)] = rslot[blk == b] % 128
        real = perm_e >= 0
        psafe = np.where(real, perm_e, 0)

        # edge feats transposed [8, E_CAP], zeros for dummies
        efT = np.where(real[None, :], ef[psafe].T, 0.0).astype(BF16)
        # per-edge scalars [y0,y10,y11,y12] tiled: [128, TILES*4]
        ya = np.where(real[:, None], ea[psafe], 0.0).astype(np.float32)
        yq = ya.reshape(TILES, 128, 4).transpose(1, 0, 2).reshape(128, TILES * 4)
        yq = yq.astype(np.float32)
        # sender indices wrapped into 16 partitions, replicated to 128
        snd = np.where(real, send[psafe], 0).astype(np.int16)
        w16 = snd.reshape(E_CAP // 16, 16).T                      # [16, E_CAP/16]
        sidx = np.tile(w16, (8, 1)).copy()                        # [128, E_CAP/16]
        # one-hot S1 [E_CAP, 128] -> resident layout [128, TILES*128]
        S1 = np.zeros((E_CAP, 128), np.float32)
        S1[np.arange(E_CAP), rcol] = 1.0
        S1[~real] = 0.0
        S1 = S1.reshape(TILES, 128, 128).transpose(1, 0, 2).reshape(
            128, TILES * 128).astype(BF16)

        in_maps.append(dict(
            efT=efT, yq=yq, sidx=sidx, S1=S1, nfT=nfT,
            w1=w1, w2=w2, w3=w3, w4=w4,
            wup=wup.reshape(256, 128),
            wtail=wtail.reshape(A * 4 * 128, 128),
            ident=ident,
        ))
    _LAYOUT["perms"] = perms
    return in_maps


def _assemble_output(results):
    """per-core 'out' [4, 128, LNPAD] bf16/f32 -> full [N, 512] f32."""
    out = np.zeros((N, 4 * C), np.float32)
    perms = _LAYOUT["perms"]
    for m in range(NCORES):
        o = np.asarray(results[m]["out"], np.float32)   # [4,128,LNPAD]
        perm = perms[m]
        live = perm >= 0
        slots = np.nonzero(live)[0]
        orig = perm[slots] + m * NPC
        out[orig, :C] = o[0][:, slots].T
        for i in range(3):
            out[orig, C + i::3] = o[1 + i][:, slots].T
    return out


# ---------------------------------------------------------------------------
# Device kernel
# ---------------------------------------------------------------------------


def _build_nc():
    import os
    from concourse import bass, bacc, tile, mybir

    dt = mybir.dt
    AF = mybir.ActivationFunctionType
    OP = mybir.AluOpType
    ITERS = int(os.environ.get("KITERS", "1"))

    caps = _LAYOUT["caps"]
    TILES = _LAYOUT["TILES"]
    E_CAP = _LAYOUT["E_CAP"]
    tile0 = [0]
    for c in caps:
        tile0.append(tile0[-1] + c)

    nc = bacc.Bacc("TRN2", target_bir_lowering=False, debug=False,
                   num_devices=NCORES)

    d_efT = nc.dram_tensor("efT", [8, E_CAP], dt.bfloat16, kind="ExternalInput")
    d_yq = nc.dram_tensor("yq", [128, TILES * 4], dt.float32, kind="ExternalInput")
    d_sidx = nc.dram_tensor("sidx", [128, E_CAP // 16], dt.int16, kind="ExternalInput")
    d_S1 = nc.dram_tensor("S1", [128, TILES * 128], dt.bfloat16, kind="ExternalInput")
    d_nfT = nc.dram_tensor("nfT", [512, N_PAD], dt.bfloat16, kind="ExternalInput")
    d_w1 = nc.dram_tensor("w1", [8, 64], dt.bfloat16, kind="ExternalInput")
    d_w2 = nc.dram_tensor("w2", [64, 64], dt.bfloat16, kind="ExternalInput")
    d_w3 = nc.dram_tensor("w3", [64, 64], dt.bfloat16, kind="ExternalInput")
    d_w4 = nc.dram_tensor("w4", [64, 512], dt.bfloat16, kind="ExternalInput")
    d_wup = nc.dram_tensor("wup", [256, 128], dt.bfloat16, kind="ExternalInput")
    d_wtail = nc.dram_tensor("wtail", [A * 4 * 128, 128], dt.bfloat16,
                             kind="ExternalInput")
    d_ident = nc.dram_tensor("ident", [128, 128], dt.bfloat16, kind="ExternalInput")
    d_out = nc.dram_tensor("out", [4, 128, LNPAD], dt.bfloat16, kind="ExternalOutput")
    d_tab = [nc.dram_tensor(f"table{i}", [N_PAD, 512], dt.bfloat16, kind="Internal")
             for i in range(min(2, ITERS))]

    with tile.TileContext(nc) as tc:
        with (
            tc.tile_pool(name="const", bufs=1) as cpool,
            tc.tile_pool(name="work", bufs=4) as wpool,
            tc.tile_pool(name="gbuf", bufs=3) as gpool,
            tc.tile_pool(name="upool", bufs=4) as upool,
            tc.tile_pool(name="msg", bufs=4) as mpool,
            tc.tile_pool(name="sy", bufs=4) as sypool,
            tc.tile_pool(name="blk", bufs=2) as bpool,
            tc.tile_pool(name="psW", bufs=2, space=bass.MemorySpace.PSUM) as psW,
            tc.tile_pool(name="psH", bufs=2, space=bass.MemorySpace.PSUM) as psH,
            tc.tile_pool(name="psM", bufs=1, space=bass.MemorySpace.PSUM) as psM,
        ):
            # ---- resident constants / static data ----
            yq = cpool.tile([128, TILES * 4], dt.float32)
            nc.sync.dma_start(yq[:], d_yq[:])
            sidx = cpool.tile([128, E_CAP // 16], dt.int16)
            nc.sync.dma_start(sidx[:], d_sidx[:])
            S1 = cpool.tile([128, TILES * 128], dt.bfloat16)
            nc.sync.dma_start(S1[:], d_S1[:])
            w1 = cpool.tile([72, 64], dt.bfloat16)
            nc.sync.dma_start(w1[0:8, :], d_w1[:])
            nc.sync.dma_start(w1[64:72, :], d_w1[:])
            w2 = cpool.tile([128, 64], dt.bfloat16)
            nc.sync.dma_start(w2[0:64, :], d_w2[:])
            nc.sync.dma_start(w2[64:128, :], d_w2[:])
            w3 = cpool.tile([128, 64], dt.bfloat16)
            nc.sync.dma_start(w3[0:64, :], d_w3[:])
            nc.sync.dma_start(w3[64:128, :], d_w3[:])
            w4 = cpool.tile([128, 512], dt.bfloat16)
            nc.sync.dma_start(w4[0:64, :], d_w4[:])
            nc.sync.dma_start(w4[64:128, :], d_w4[:])
            wup = cpool.tile([128, 256], dt.bfloat16)
            nc.sync.dma_start(wup[:].rearrange("p (k c) -> p k c", k=2),
                              d_wup[:].rearrange("(k p) c -> p k c", k=2))
            wtail = cpool.tile([128, A * 4 * 128], dt.bfloat16)
            nc.sync.dma_start(wtail[:].rearrange("p (k c) -> p k c", k=A * 4),
                              d_wtail[:].rearrange("(k p) c -> p k c", k=A * 4))
            ident = cpool.tile([128, 128], dt.bfloat16)
            nc.sync.dma_start(ident[:], d_ident[:])

            for it in range(ITERS):
                d_table = d_tab[it % len(d_tab)]
                # ---- Phase 1: linear_up table, streamed slabs of 8 node tiles
                G8 = 8
                ngrp = (NT_UP + G8 - 1) // G8
                for g in range(ngrp):
                    nts = list(range(g * G8, min((g + 1) * G8, NT_UP)))
                    wcols = len(nts) * 128
                    slabs = []
                    for comp in range(4):
                        slab = upool.tile([128, G8 * 128], dt.bfloat16, tag="upslab")
                        nc.sync.dma_start(
                            slab[:, :wcols],
                            d_nfT[comp * 128:(comp + 1) * 128,
                                  nts[0] * 128: nts[0] * 128 + wcols])
                        slabs.append(slab)
                    stage = upool.tile([128, G8 * 512], dt.bfloat16,
                                       tag="upstage", bufs=2)
                    for j, nt in enumerate(nts):
                        ps = psH.tile([128, 512], dt.float32, tag="psH")
                        for comp in range(4):
                            nc.tensor.matmul(
                                ps[:, comp * 128:(comp + 1) * 128],
                                slabs[comp][:, j * 128:(j + 1) * 128],
                                wup[:, (0 if comp == 0 else 128):
                                    (128 if comp == 0 else 256)],
                                start=True, stop=True)
                        nc.any.tensor_copy(stage[:, j * 512:(j + 1) * 512],
                                           ps[:])
                    nc.sync.dma_start(
                        d_table[nts[0] * 128: nts[0] * 128 + wcols, :].rearrange(
                            "(t p) e -> p t e", p=128),
                        stage[:, :len(nts) * 512].rearrange(
                            "p (t e) -> p t e", e=512))

                # ---- Phase 2: MLP + messages + scatter, per block ----
                for b in range(BLOCKS):
                    nt_b = caps[b]
                    t_lo = tile0[b]
                    e_lo = t_lo * 128

                    macc = psM.tile([128, 2048], dt.float32, tag="macc")

                    # chunks of <=8 tiles: gather + edge feats + MLP + messages
                    done = 0   # tiles of this block processed
                    while done < nt_b:
                        tch = min(8, nt_b - done)
                        ta = (tch + 1) // 2
                        tb = tch - ta
                        e0 = (t_lo + done) * 128   # global edge offset
                        wa, wb = ta * 128, tb * 128
                        gb = gpool.tile([128, 8 * 512], dt.bfloat16, tag="gather")
                        nc.gpsimd.dma_gather(
                            out_ap=gb[:, :tch * 512].rearrange(
                                "p (t e) -> p t e", e=512),
                            in_ap=d_table[:, :],
                            idxs_ap=sidx[:, e0 // 16:(e0 + tch * 128) // 16],
                            num_idxs=tch * 128, num_idxs_reg=tch * 128,
                            elem_size=512,
                        )
                        efb = wpool.tile([72, 8 * 128], dt.bfloat16, tag="efb")
                        nc.sync.dma_start(efb[0:8, :tch * 128],
                                          d_efT[:, e0:e0 + tch * 128])
                        nc.sync.dma_start(efb[64:72, :tch * 128],
                                          d_efT[:, e0:e0 + tch * 128])

                        h = psH.tile([128, 512], dt.float32, tag="psH")
                        nc.tensor.matmul(h[0:64, :wa], w1[0:8, :],
                                         efb[0:8, 0:wa],
                                         start=True, stop=True)
                        if tb:
                            nc.tensor.matmul(h[64:128, :wb], w1[64:72, :],
                                             efb[64:72, wa:wa + wb],
                                             start=True, stop=True)
                        h1 = wpool.tile([128, 512], dt.bfloat16, tag="h1")
                        if ta == tb:
                            nc.scalar.activation(h1[:, :wa], h[:, :wa], AF.Silu)
                        else:
                            nc.scalar.activation(h1[0:64, :wa], h[0:64, :wa],
                                                 AF.Silu)
                            if tb:
                                nc.scalar.activation(h1[64:128, :wb],
                                                     h[64:128, :wb], AF.Silu)
                        h = psH.tile([128, 512], dt.float32, tag="psH")
                        nc.tensor.matmul(h[0:64, :wa], w2[0:64, :], h1[0:64, :wa],
                                         start=True, stop=True)
                        if tb:
                            nc.tensor.matmul(h[64:128, :wb], w2[64:128, :],
                                             h1[64:128, :wb],
                                             start=True, stop=True)
                        h2 = wpool.tile([128, 512], dt.bfloat16, tag="h2")
                        if ta == tb:
                            nc.scalar.activation(h2[:, :wa], h[:, :wa], AF.Silu)
                        else:
                            nc.scalar.activation(h2[0:64, :wa], h[0:64, :wa],
                                                 AF.Silu)
                            if tb:
                                nc.scalar.activation(h2[64:128, :wb],
                                                     h[64:128, :wb], AF.Silu)
                        h = psH.tile([128, 512], dt.float32, tag="psH")
                        nc.tensor.matmul(h[0:64, :wa], w3[0:64, :], h2[0:64, :wa],
                                         start=True, stop=True)
                        if tb:
                            nc.tensor.matmul(h[64:128, :wb], w3[64:128, :],
                                             h2[64:128, :wb],
                                             start=True, stop=True)
                        h3 = wpool.tile([128, 512], dt.bfloat16, tag="h3")
                        if ta == tb:
                            nc.scalar.activation(h3[:, :wa], h[:, :wa], AF.Silu)
                        else:
                            nc.scalar.activation(h3[0:64, :wa], h[0:64, :wa],
                                                 AF.Silu)
                            if tb:
                                nc.scalar.activation(h3[64:128, :wb],
                                                     h[64:128, :wb], AF.Silu)

                        for s in range(tch):
                            k = done + s           # tile within block
                            t = t_lo + k           # global tile
                            hh = 0 if s < ta else 64
                            hcol = (s if s < ta else s - ta) * 128
                            wt_ps = psW.tile([128, 512], dt.float32, tag="psW")
                            nc.tensor.matmul(
                                wt_ps[:], h3[hh:hh + 64, hcol:hcol + 128],
                                w4[hh:hh + 64, :], start=True, stop=True)
                            # wt -> SBUF bf16 (y0 via Sy0 stationary)
                            wts = wpool.tile([128, 512], dt.bfloat16, tag="wts")
                            nc.any.tensor_copy(wts[:], wt_ps[:])

                            Gt = gb[:, s * 512:(s + 1) * 512]
                            # messages: plain tensor_tensor (2x bf16 mode)
                            # [m1|mv2] = [ws1y0*s | wv2y0*v_i]; [u|b] = [wv1*s | ws2*v_i]
                            msg = mpool.tile([128, 1024], dt.bfloat16, tag="msg")
                            nc.vector.tensor_tensor(
                                msg[:, 0:128], Gt[:, 0:128], wts[:, 0:128],
                                OP.mult)
                            nc.vector.tensor_tensor(
                                msg[:, 128:512].rearrange("p (i c) -> p i c", i=3),
                                Gt[:, 128:512].rearrange("p (i c) -> p i c", i=3),
                                wts[:, 128:256].unsqueeze(1).broadcast_to(
                                    (128, 3, 128)),
                                OP.mult)
                            nc.gpsimd.tensor_tensor(
                                msg[:, 512:640], Gt[:, 0:128], wts[:, 384:512],
                                OP.mult)
                            nc.vector.tensor_tensor(
                                msg[:, 640:1024].rearrange("p (i c) -> p i c", i=3),
                                Gt[:, 128:512].rearrange("p (i c) -> p i c", i=3),
                                wts[:, 256:384].unsqueeze(1).broadcast_to(
                                    (128, 3, 128)),
                                OP.mult)

                            # scaled one-hot stationaries Sy0..Sy3 = S1 * y
                            # Sy0/Sy1/Sy3 on DVE (cheap ts), Sy2 on Pool
                            S1t = S1[:, t * 128:(t + 1) * 128]
                            sy = sypool.tile([128, 512], dt.bfloat16, tag="sy")
                            nc.vector.tensor_scalar_mul(
                                sy[:, 0:128], S1t, yq[:, t * 4:t * 4 + 1])
                            nc.vector.tensor_scalar_mul(
                                sy[:, 128:256], S1t, yq[:, t * 4 + 1:t * 4 + 2])
                            nc.gpsimd.tensor_scalar_mul(
                                sy[:, 256:384], S1t, yq[:, t * 4 + 2:t * 4 + 3])
                            nc.vector.tensor_scalar_mul(
                                sy[:, 384:512], S1t, yq[:, t * 4 + 3:t * 4 + 4])

                            st, sp = (k == 0), (k == nt_b - 1)
                            # A1|C0|C1|C2 <- Sy0^T [m1|mv2]
                            nc.tensor.matmul(macc[:, 0:512], sy[:, 0:128],
                                             msg[:, 0:512],
                                             start=st, stop=sp,
                                             skip_group_check=True)
                            ub = msg[:, 512:1024].rearrange(
                                "p (g c) -> p g c", c=128)
                            for i in range(3):
                                # [B_i | A2_i] <- Syi^T [u | b_i]  (own bank)
                                nc.tensor.matmul(
                                    macc[:, 512 + 512 * i:768 + 512 * i],
                                    sy[:, 128 * (1 + i):128 * (2 + i)],
                                    ub[:, 0:(2 + i):(1 + i), :],
                                    start=st, stop=sp, skip_group_check=True)
                        done += tch

                    # ---- block tail: transpose 8 chunks, fused mid+skip ----
                    # msb = [A1|C0|C1|C2 | B0|B1|B2 | A2sum]
                    msb = bpool.tile([128, 1024], dt.bfloat16, tag="msb")
                    nc.any.tensor_copy(msb[:, 0:640], macc[:, 0:640])
                    nc.any.tensor_copy(msb[:, 640:768], macc[:, 1024:1152])
                    nc.any.tensor_copy(msb[:, 768:896], macc[:, 1536:1664])
                    a2t = bpool.tile([128, 256], dt.float32, tag="a2t")
                    nc.any.tensor_copy(a2t[:, 0:128], macc[:, 640:768])
                    nc.vector.tensor_tensor(a2t[:, 128:256], a2t[:, 0:128],
                                            macc[:, 1152:1280], OP.add)
                    nc.vector.tensor_tensor(msb[:, 896:1024], a2t[:, 128:256],
                                            macc[:, 1664:1792], OP.add)
                    TT = bpool.tile([128, 1024], dt.bfloat16, tag="TT")
                    for half in range(2):
                        tp = psH.tile([128, 512], dt.bfloat16, tag="psH")
                        for j in range(4):
                            cc = half * 512 + j * 128
                            nc.tensor.transpose(tp[:, j * 128:(j + 1) * 128],
                                                msb[:, cc:cc + 128], ident[:])
                        nc.any.tensor_copy(TT[:, half * 512:half * 512 + 512],
                                           tp[:])
                    # TT chunks: [A1|C0|C1|C2|B0|B1|B2|A2]
                    pm = psW.tile([128, 512], dt.float32, tag="psW")
                    for (a, c0, c1) in _RUNS[b]:
                        wslab = wtail[:, a * 512:(a + 1) * 512]
                        # out_s <- FS1^T A1 + FS2^T A2
                        nc.tensor.matmul(pm[:, c0:c1], wslab[:, 0:128],
                                         TT[:, c0:c1], start=True, stop=False,
                                         skip_group_check=True)
                        nc.tensor.matmul(pm[:, c0:c1], wslab[:, 128:256],
                                         TT[:, 896 + c0:896 + c1],
                                         start=False, stop=True,
                                         skip_group_check=True)
                        for i in range(3):
                            oc = 128 * (1 + i)
                            nc.tensor.matmul(
                                pm[:, oc + c0:oc + c1], wslab[:, 256:384],
                                TT[:, 512 + 128 * i + c0:512 + 128 * i + c1],
                                start=True, stop=False, skip_group_check=True)
                            nc.tensor.matmul(
                                pm[:, oc + c0:oc + c1], wslab[:, 384:512],
                                TT[:, 128 * (1 + i) + c0:128 * (1 + i) + c1],
                                start=False, stop=True, skip_group_check=True)
                    ob = bpool.tile([128, 512], dt.bfloat16, tag="ob")
                    nc.any.tensor_copy(ob[:], pm[:])
                    nc.sync.dma_start(
                        d_out[:, :, b * 128:(b + 1) * 128].rearrange(
                            "x p n -> p x n"),
                        ob[:].rearrange("p (x n) -> p x n", x=4))

    nc.compile()
    return nc


def kernel(**inputs):
    in_maps = _prep_host(**inputs)
    key = _LAYOUT["caps"]
    if key not in _CACHE:
        _CACHE[key] = _build_nc()
    nc = _CACHE[key]
    from concourse.bass_utils import run_bass_kernel_spmd
    res = run_bass_kernel_spmd(nc, in_maps, core_ids=list(range(NCORES)))
    return _assemble_output(res.results)


# revision 19
# speedup vs baseline: 2.3968x; 2.3968x over previous
"""Trainium2 Bass kernel for AgnosticNonlinearInteractionBlock (GNN message passing).

Sharding: edges partitioned by receiver across 8 cores; each core computes full
output rows for its 1250-node slice. No collectives.

v3 design (vs baseline):
  - receiver nodes attr-sorted into static G=160 slot ranges -> the mid linear
    and skip_tp fuse into per-attr [2C->C] weights applied per static attr-run
    (eliminates the dense skip_tp phase entirely)
  - per-edge y0/y1i factors folded into on-device-built scaled one-hot scatter
    stationaries Sy0..Sy3 (= S1 * y); messages become plain tensor_tensor
    products (2x DVE mode), wt read directly from PSUM
  - S1 / sidx / yq resident in SBUF (loaded once per NEFF, not per iteration)
  - per-block tile counts data-driven (balanced within attr ranges)
  - d_table double-buffered across iterations so KITERS>1 pipelines
"""

import sys

sys.path.insert(0, "/opt/trn_rl_repo")

import numpy as np
import ml_dtypes

BF16 = ml_dtypes.bfloat16

# Problem constants
N, E = 10000, 160000
C, A, R, H = 128, 10, 8, 64
AVG_NEI = 16.0
INV_SQRT3 = 1.0 / np.sqrt(3.0)

NCORES = 8
NPC = N // NCORES          # 1250 local nodes per core
G_ATTR = 160               # static slots per attr value
BLOCKS = (G_ATTR * A + 127) // 128   # 13 receiver blocks of 128 slots
LNPAD = BLOCKS * 128       # 1664 local padded node slots
N_PAD = 10112              # 79*128 padded node count for the up table
NT_UP = N_PAD // 128
MLP_CH = 384               # mlp chunk = 3 tiles worth of columns (768 edges)

# static run table: block -> list of (attr, col0, col1); last run extended to 128
_RUNS = []
for _b in range(BLOCKS):
    lo, hi = _b * 128, _b * 128 + 128
    rs = []
    for _a in range(A):
        s0, s1 = max(lo, _a * G_ATTR), min(hi, (_a + 1) * G_ATTR)
        if s1 > s0:
            rs.append([_a, s0 - lo, s1 - lo])
    rs[-1][2] = 128
    _RUNS.append([tuple(r) for r in rs])

_LAYOUT = {}   # set by _prep_host: caps (tiles per block), TILES, E_CAP
_CACHE = {}


def _pack_core(deg_local, attr_local):
    """Assign local nodes to slots: attr a -> slots [a*G, (a+1)*G); within the
    blocks an attr range spans, place high-degree nodes into the least edge-
    loaded block. Returns perm (slot -> local node id or -1) and per-block
    edge loads."""
    perm = np.full(LNPAD, -1, np.int64)
    load = np.zeros(BLOCKS)
    for a in range(A):
        nodes = np.where(attr_local == a)[0]
        assert len(nodes) <= G_ATTR, f"attr {a}: {len(nodes)} > {G_ATTR}"
        lo, hi = a * G_ATTR, (a + 1) * G_ATTR
        segs = []   # (block, slot0, nslots)
        for k in range(lo // 128, (hi - 1) // 128 + 1):
            s0, s1 = max(lo, 128 * k), min(hi, 128 * (k + 1))
            segs.append([k, s0, s1 - s0])
        cnt = {k: 0 for k, _, _ in segs}
        cap = {k: n for k, _, n in segs}
        s0of = {k: s for k, s, _ in segs}
        for n in nodes[np.argsort(-deg_local[nodes])]:
            k = min((k for k in cap if cnt[k] < cap[k]), key=lambda k: load[k])
            perm[s0of[k] + cnt[k]] = n
            cnt[k] += 1
            load[k] += deg_local[n]
    return perm, load


def _prep_host(node_attrs, node_feats, edge_attrs, edge_feats, edge_index,
               W_up0, W_up1, W_mlp1, W_mlp2, W_mlp3, W_mlp4,
               W_lin0, W_lin1, W_skip0, W_skip1):
    send = np.asarray(edge_index[0]).astype(np.int64)
    recv = np.asarray(edge_index[1]).astype(np.int64)
    ef = np.asarray(edge_feats, np.float32)
    ea = np.asarray(edge_attrs, np.float32)
    attr = np.asarray(node_attrs).argmax(1)
    deg = np.bincount(recv, minlength=N)

    # ---- per-core node permutation + per-block tile caps (core-invariant) ----
    perms, loads = [], []
    for m in range(NCORES):
        lo = m * NPC
        p, ld = _pack_core(deg[lo:lo + NPC], attr[lo:lo + NPC])
        perms.append(p)
        loads.append(ld)
    caps = np.ceil(np.stack(loads).max(0) / 128.0).astype(int) + 0
    caps = np.maximum(caps, 1)
    TILES = int(caps.sum())
    E_CAP = TILES * 128
    _LAYOUT["caps"] = tuple(int(c) for c in caps)
    _LAYOUT["TILES"] = TILES
    _LAYOUT["E_CAP"] = E_CAP
    tile0 = np.concatenate([[0], np.cumsum(caps)])  # first tile of block b

    # ---- shared weights ----
    w1 = (np.asarray(W_mlp1, np.float32) / np.sqrt(R)).astype(BF16)
    w2 = (np.asarray(W_mlp2, np.float32) / np.sqrt(H)).astype(BF16)
    w3 = (np.asarray(W_mlp3, np.float32) / np.sqrt(H)).astype(BF16)
    # w4 column-reordered: [ws1 | wv2 | ws2*inv_sqrt3 | wv1]
    w4f = np.asarray(W_mlp4, np.float32) / np.sqrt(H)
    w4 = np.concatenate([w4f[:, 0:C], w4f[:, 3 * C:4 * C],
                         w4f[:, C:2 * C] * INV_SQRT3, w4f[:, 2 * C:3 * C]],
                        axis=1).astype(BF16)                       # [64, 512]
    wup = (np.stack([np.asarray(W_up0, np.float32), np.asarray(W_up1, np.float32)])
           / np.sqrt(C)).astype(BF16)                             # [2,128,128]

    # fused tail weights per attr: [FS1|FS2|FV1|FV2], scale folded
    norm = np.sqrt(2 * C) * AVG_NEI
    fan = np.sqrt(C * A)
    sc = 1.0 / (norm * fan)
    wl0 = np.asarray(W_lin0, np.float64)
    wl1 = np.asarray(W_lin1, np.float64)
    wsk0 = np.asarray(W_skip0, np.float64)
    wsk1 = np.asarray(W_skip1, np.float64)
    wtail = np.zeros((A, 4, 128, 128), np.float64)
    for a in range(A):
        wtail[a, 0] = wl0[:C] @ wsk0[:, a, :] * sc
        wtail[a, 1] = wl0[C:] @ wsk0[:, a, :] * sc
        wtail[a, 2] = wl1[:C] @ wsk1[:, a, :] * sc
        wtail[a, 3] = wl1[C:] @ wsk1[:, a, :] * sc
    wtail = wtail.astype(BF16)                                    # [A,4,128,128]

    # node_feats transposed planes for phase 1: [4,128,N_PAD] -> [512, N_PAD]
    nfT = np.zeros((4, 128, N_PAD), np.float32)
    nfT[0, :, :N] = np.asarray(node_feats, np.float32)[:, :C].T
    v = np.asarray(node_feats, np.float32)[:, C:].reshape(N, C, 3)
    for i in range(3):
        nfT[1 + i, :, :N] = v[:, :, i].T
    nfT = nfT.reshape(512, N_PAD).astype(BF16)

    ident = np.eye(128, dtype=BF16)

    in_maps = []
    for m in range(NCORES):
        lo = m * NPC
        perm = perms[m]
        # slot of each local node
        slot_of = np.full(NPC, -1, np.int64)
        live = perm >= 0
        slot_of[perm[live]] = np.nonzero(live)[0]

        # bucket edges into blocks by receiver slot
        mask = (recv >= lo) & (recv < lo + NPC)
        eidx = np.nonzero(mask)[0]
        rslot = slot_of[recv[eidx] - lo]
        blk = rslot // 128
        perm_e = np.full(E_CAP, -1, np.int64)
        rcol = np.zeros(E_CAP, np.int64)
        for b in range(BLOCKS):
            be = eidx[blk == b]
            cap = caps[b] * 128
            assert len(be) <= cap, f"core {m} block {b}: {len(be)} > {cap}"
            o = tile0[b] * 128
            perm_e[o:o + len(be)] = be
            rcol[o:o + len(be)] = rslot[blk == b] % 128
        real = perm_e >= 0
        psafe = np.where(real, perm_e, 0)

        # edge feats transposed [8, E_CAP], zeros for dummies
        efT = np.where(real[None, :], ef[psafe].T, 0.0).astype(BF16)
        # per-edge scalars [y0,y10,y11,y12] tiled: [128, TILES*4]
        ya = np.where(real[:, None], ea[psafe], 0.0).astype(np.float32)
        yq = ya.reshape(TILES, 128, 4).transpose(1, 0, 2).reshape(128, TILES * 4)
        yq = yq.astype(np.float32)
        # sender indices wrapped into 16 partitions, replicated to 128
        snd = np.where(real, send[psafe], 0).astype(np.int16)
        w16 = snd.reshape(E_CAP // 16, 16).T                      # [16, E_CAP/16]
        sidx = np.tile(w16, (8, 1)).copy()                        # [128, E_CAP/16]
        # one-hot S1 [E_CAP, 128] -> resident layout [128, TILES*128]
        S1 = np.zeros((E_CAP, 128), np.float32)
        S1[np.arange(E_CAP), rcol] = 1.0
        S1[~real] = 0.0
        S1 = S1.reshape(TILES, 128, 128).transpose(1, 0, 2).reshape(
            128, TILES * 128).astype(BF16)

        in_maps.append(dict(
            efT=efT, yq=yq, sidx=sidx, S1=S1, nfT=nfT,
            w1=w1, w2=w2, w3=w3, w4=w4,
            wup=wup.reshape(256, 128),
            wtail=wtail.reshape(A * 4 * 128, 128),
            ident=ident,
        ))
    _LAYOUT["perms"] = perms
    return in_maps


def _assemble_output(results):
    """per-core 'out' [4, 128, LNPAD] bf16/f32 -> full [N, 512] f32."""
    out = np.zeros((N, 4 * C), np.float32)
    perms = _LAYOUT["perms"]
    for m in range(NCORES):
        o = np.asarray(results[m]["out"], np.float32)   # [4,128,LNPAD]
        perm = perms[m]
        live = perm >= 0
        slots = np.nonzero(live)[0]
        orig = perm[slots] + m * NPC
        out[orig, :C] = o[0][:, slots].T
        for i in range(3):
            out[orig, C + i::3] = o[1 + i][:, slots].T
    return out


# ---------------------------------------------------------------------------
# Device kernel
# ---------------------------------------------------------------------------


def _build_nc():
    import os
    from concourse import bass, bacc, tile, mybir

    dt = mybir.dt
    AF = mybir.ActivationFunctionType
    OP = mybir.AluOpType
    ITERS = int(os.environ.get("KITERS", "1"))

    caps = _LAYOUT["caps"]
    TILES = _LAYOUT["TILES"]
    E_CAP = _LAYOUT["E_CAP"]
    tile0 = [0]
    for c in caps:
        tile0.append(tile0[-1] + c)

    nc = bacc.Bacc("TRN2", target_bir_lowering=False, debug=False,
                   num_devices=NCORES)

    d_efT = nc.dram_tensor("efT", [8, E_CAP], dt.bfloat16, kind="ExternalInput")
    d_yq = nc.dram_tensor("yq", [128, TILES * 4], dt.float32, kind="ExternalInput")
    d_sidx = nc.dram_tensor("sidx", [128, E_CAP // 16], dt.int16, kind="ExternalInput")
    d_S1 = nc.dram_tensor("S1", [128, TILES * 128], dt.bfloat16, kind="ExternalInput")
    d_nfT = nc.dram_tensor("nfT", [512, N_PAD], dt.bfloat16, kind="ExternalInput")
    d_w1 = nc.dram_tensor("w1", [8, 64], dt.bfloat16, kind="ExternalInput")
    d_w2 = nc.dram_tensor("w2", [64, 64], dt.bfloat16, kind="ExternalInput")
    d_w3 = nc.dram_tensor("w3", [64, 64], dt.bfloat16, kind="ExternalInput")
    d_w4 = nc.dram_tensor("w4", [64, 512], dt.bfloat16, kind="ExternalInput")
    d_wup = nc.dram_tensor("wup", [256, 128], dt.bfloat16, kind="ExternalInput")
    d_wtail = nc.dram_tensor("wtail", [A * 4 * 128, 128], dt.bfloat16,
                             kind="ExternalInput")
    d_ident = nc.dram_tensor("ident", [128, 128], dt.bfloat16, kind="ExternalInput")
    d_out = nc.dram_tensor("out", [4, 128, LNPAD], dt.bfloat16, kind="ExternalOutput")
    d_tab = [nc.dram_tensor(f"table{i}", [N_PAD, 512], dt.bfloat16, kind="Internal")
             for i in range(min(2, ITERS))]

    with tile.TileContext(nc) as tc:
        with (
            tc.tile_pool(name="const", bufs=1) as cpool,
            tc.tile_pool(name="work", bufs=4) as wpool,
            tc.tile_pool(name="gbuf", bufs=3) as gpool,
            tc.tile_pool(name="upool", bufs=4) as upool,
            tc.tile_pool(name="msg", bufs=4) as mpool,
            tc.tile_pool(name="sy", bufs=4) as sypool,
            tc.tile_pool(name="blk", bufs=2) as bpool,
            tc.tile_pool(name="psW", bufs=2, space=bass.MemorySpace.PSUM) as psW,
            tc.tile_pool(name="psH", bufs=2, space=bass.MemorySpace.PSUM) as psH,
            tc.tile_pool(name="psM", bufs=1, space=bass.MemorySpace.PSUM) as psM,
        ):
            # ---- resident constants / static data ----
            yq = cpool.tile([128, TILES * 4], dt.float32)
            nc.sync.dma_start(yq[:], d_yq[:])
            sidx = cpool.tile([128, E_CAP // 16], dt.int16)
            nc.sync.dma_start(sidx[:], d_sidx[:])
            S1 = cpool.tile([128, TILES * 128], dt.bfloat16)
            nc.sync.dma_start(S1[:], d_S1[:])
            w1 = cpool.tile([72, 64], dt.bfloat16)
            nc.sync.dma_start(w1[0:8, :], d_w1[:])
            nc.sync.dma_start(w1[64:72, :], d_w1[:])
            w2 = cpool.tile([128, 64], dt.bfloat16)
            nc.sync.dma_start(w2[0:64, :], d_w2[:])
            nc.sync.dma_start(w2[64:128, :], d_w2[:])
            w3 = cpool.tile([128, 64], dt.bfloat16)
            nc.sync.dma_start(w3[0:64, :], d_w3[:])
            nc.sync.dma_start(w3[64:128, :], d_w3[:])
            w4 = cpool.tile([128, 512], dt.bfloat16)
            nc.sync.dma_start(w4[0:64, :], d_w4[:])
            nc.sync.dma_start(w4[64:128, :], d_w4[:])
            wup = cpool.tile([128, 256], dt.bfloat16)
            nc.sync.dma_start(wup[:].rearrange("p (k c) -> p k c", k=2),
                              d_wup[:].rearrange("(k p) c -> p k c", k=2))
            wtail = cpool.tile([128, A * 4 * 128], dt.bfloat16)
            nc.sync.dma_start(wtail[:].rearrange("p (k c) -> p k c", k=A * 4),
                              d_wtail[:].rearrange("(k p) c -> p k c", k=A * 4))
            ident = cpool.tile([128, 128], dt.bfloat16)
            nc.sync.dma_start(ident[:], d_ident[:])

            for it in range(ITERS):
                d_table = d_tab[it % len(d_tab)]
                # ---- Phase 1: linear_up table, streamed slabs of 8 node tiles
                G8 = 8
                ngrp = (NT_UP + G8 - 1) // G8
                for g in range(ngrp):
                    nts = list(range(g * G8, min((g + 1) * G8, NT_UP)))
                    wcols = len(nts) * 128
                    slabs = []
                    for comp in range(4):
                        slab = upool.tile([128, G8 * 128], dt.bfloat16, tag="upslab")
                        nc.sync.dma_start(
                            slab[:, :wcols],
                            d_nfT[comp * 128:(comp + 1) * 128,
                                  nts[0] * 128: nts[0] * 128 + wcols])
                        slabs.append(slab)
                    stage = upool.tile([128, G8 * 512], dt.bfloat16,
                                       tag="upstage", bufs=2)
                    for j, nt in enumerate(nts):
                        ps = psH.tile([128, 512], dt.float32, tag="psH")
                        for comp in range(4):
                            nc.tensor.matmul(
                                ps[:, comp * 128:(comp + 1) * 128],
                                slabs[comp][:, j * 128:(j + 1) * 128],
                                wup[:, (0 if comp == 0 else 128):
                                    (128 if comp == 0 else 256)],
                                start=True, stop=True)
                        nc.any.tensor_copy(stage[:, j * 512:(j + 1) * 512],
                                           ps[:])
                    nc.sync.dma_start(
                        d_table[nts[0] * 128: nts[0] * 128 + wcols, :].rearrange(
                            "(t p) e -> p t e", p=128),
                        stage[:, :len(nts) * 512].rearrange(
                            "p (t e) -> p t e", e=512))

                # ---- Phase 2: MLP + messages + scatter, per block ----
                for b in range(BLOCKS):
                    nt_b = caps[b]
                    t_lo = tile0[b]
                    e_lo = t_lo * 128

                    macc = psM.tile([128, 2048], dt.float32, tag="macc")

                    # chunks of <=8 tiles: gather + edge feats + MLP + messages
                    done = 0   # tiles of this block processed
                    while done < nt_b:
                        tch = min(8, nt_b - done)
                        ta = (tch + 1) // 2
                        tb = tch - ta
                        e0 = (t_lo + done) * 128   # global edge offset
                        wa, wb = ta * 128, tb * 128
                        gb = gpool.tile([128, 8 * 512], dt.bfloat16, tag="gather")
                        nc.gpsimd.dma_gather(
                            out_ap=gb[:, :tch * 512].rearrange(
                                "p (t e) -> p t e", e=512),
                            in_ap=d_table[:, :],
                            idxs_ap=sidx[:, e0 // 16:(e0 + tch * 128) // 16],
                            num_idxs=tch * 128, num_idxs_reg=tch * 128,
                            elem_size=512,
                        )
                        efb = wpool.tile([72, 8 * 128], dt.bfloat16, tag="efb")
                        nc.sync.dma_start(efb[0:8, :tch * 128],
                                          d_efT[:, e0:e0 + tch * 128])
                        nc.sync.dma_start(efb[64:72, :tch * 128],
                                          d_efT[:, e0:e0 + tch * 128])

                        h = psH.tile([128, 512], dt.float32, tag="psH")
                        nc.tensor.matmul(h[0:64, :wa], w1[0:8, :],
                                         efb[0:8, 0:wa],
                                         start=True, stop=True)
                        if tb:
                            nc.tensor.matmul(h[64:128, :wb], w1[64:72, :],
                                             efb[64:72, wa:wa + wb],
                                             start=True, stop=True)
                        h1 = wpool.tile([128, 512], dt.bfloat16, tag="h1")
                        if ta == tb:
                            nc.scalar.activation(h1[:, :wa], h[:, :wa], AF.Silu)
                        else:
                            nc.scalar.activation(h1[0:64, :wa], h[0:64, :wa],
                                                 AF.Silu)
                            if tb:
                                nc.scalar.activation(h1[64:128, :wb],
                                                     h[64:128, :wb], AF.Silu)
                        h = psH.tile([128, 512], dt.float32, tag="psH")
                        nc.tensor.matmul(h[0:64, :wa], w2[0:64, :], h1[0:64, :wa],
                                         start=True, stop=True)
                        if tb:
                            nc.tensor.matmul(h[64:128, :wb], w2[64:128, :],
                                             h1[64:128, :wb],
                                             start=True, stop=True)
                        h2 = wpool.tile([128, 512], dt.bfloat16, tag="h2")
                        if ta == tb:
                            nc.scalar.activation(h2[:, :wa], h[:, :wa], AF.Silu)
                        else:
                            nc.scalar.activation(h2[0:64, :wa], h[0:64, :wa],
                                                 AF.Silu)
                            if tb:
                                nc.scalar.activation(h2[64:128, :wb],
                                                     h[64:128, :wb], AF.Silu)
                        h = psH.tile([128, 512], dt.float32, tag="psH")
                        nc.tensor.matmul(h[0:64, :wa], w3[0:64, :], h2[0:64, :wa],
                                         start=True, stop=True)
                        if tb:
                            nc.tensor.matmul(h[64:128, :wb], w3[64:128, :],
                                             h2[64:128, :wb],
                                             start=True, stop=True)
                        h3 = wpool.tile([128, 512], dt.bfloat16, tag="h3")
                        if ta == tb:
                            nc.scalar.activation(h3[:, :wa], h[:, :wa], AF.Silu)
                        else:
                            nc.scalar.activation(h3[0:64, :wa], h[0:64, :wa],
                                                 AF.Silu)
                            if tb:
                                nc.scalar.activation(h3[64:128, :wb],
                                                     h[64:128, :wb], AF.Silu)

                        for s in range(tch):
                            k = done + s           # tile within block
                            t = t_lo + k           # global tile
                            hh = 0 if s < ta else 64
                            hcol = (s if s < ta else s - ta) * 128
                            wt_ps = psW.tile([128, 512], dt.float32, tag="psW")
                            nc.tensor.matmul(
                                wt_ps[:], h3[hh:hh + 64, hcol:hcol + 128],
                                w4[hh:hh + 64, :], start=True, stop=True)
                            # wt -> SBUF bf16 (y0 via Sy0 stationary)
                            wts = wpool.tile([128, 512], dt.bfloat16, tag="wts")
                            nc.any.tensor_copy(wts[:], wt_ps[:])

                            Gt = gb[:, s * 512:(s + 1) * 512]
                            # messages: plain tensor_tensor (2x bf16 mode)
                            # [m1|mv2] = [ws1y0*s | wv2y0*v_i]; [u|b] = [wv1*s | ws2*v_i]
                            msg = mpool.tile([128, 1024], dt.bfloat16, tag="msg")
                            nc.vector.tensor_tensor(
                                msg[:, 0:128], Gt[:, 0:128], wts[:, 0:128],
                                OP.mult)
                            nc.vector.tensor_tensor(
                                msg[:, 128:512].rearrange("p (i c) -> p i c", i=3),
                                Gt[:, 128:512].rearrange("p (i c) -> p i c", i=3),
                                wts[:, 128:256].unsqueeze(1).broadcast_to(
                                    (128, 3, 128)),
                                OP.mult)
                            nc.vector.tensor_tensor(
                                msg[:, 512:640], Gt[:, 0:128], wts[:, 384:512],
                                OP.mult)
                            nc.vector.tensor_tensor(
                                msg[:, 640:1024].rearrange("p (i c) -> p i c", i=3),
                                Gt[:, 128:512].rearrange("p (i c) -> p i c", i=3),
                                wts[:, 256:384].unsqueeze(1).broadcast_to(
                                    (128, 3, 128)),
                                OP.mult)

                            # scaled one-hot stationaries Sy0..Sy3 = S1 * y
                            # Sy0/Sy1/Sy3 on DVE (cheap ts), Sy2 on Pool
                            S1t = S1[:, t * 128:(t + 1) * 128]
                            sy = sypool.tile([128, 512], dt.bfloat16, tag="sy")
                            for q in range(4):
                                nc.scalar.activation(
                                    sy[:, 128 * q:128 * (q + 1)], S1t, AF.Copy,
                                    scale=yq[:, t * 4 + q:t * 4 + q + 1])

                            st, sp = (k == 0), (k == nt_b - 1)
                            # A1|C0|C1|C2 <- Sy0^T [m1|mv2]
                            nc.tensor.matmul(macc[:, 0:512], sy[:, 0:128],
                                             msg[:, 0:512],
                                             start=st, stop=sp,
                                             skip_group_check=True)
                            ub = msg[:, 512:1024].rearrange(
                                "p (g c) -> p g c", c=128)
                            for i in range(3):
                                # [B_i | A2_i] <- Syi^T [u | b_i]  (own bank)
                                nc.tensor.matmul(
                                    macc[:, 512 + 512 * i:768 + 512 * i],
                                    sy[:, 128 * (1 + i):128 * (2 + i)],
                                    ub[:, 0:(2 + i):(1 + i), :],
                                    start=st, stop=sp, skip_group_check=True)
                        done += tch

                    # ---- block tail: transpose 8 chunks, fused mid+skip ----
                    # msb = [A1|C0|C1|C2 | B0|B1|B2 | A2sum]
                    msb = bpool.tile([128, 1024], dt.bfloat16, tag="msb")
                    nc.any.tensor_copy(msb[:, 0:640], macc[:, 0:640])
                    nc.any.tensor_copy(msb[:, 640:768], macc[:, 1024:1152])
                    nc.any.tensor_copy(msb[:, 768:896], macc[:, 1536:1664])
                    a2t = bpool.tile([128, 256], dt.float32, tag="a2t")
                    nc.any.tensor_copy(a2t[:, 0:128], macc[:, 640:768])
                    nc.vector.tensor_tensor(a2t[:, 128:256], a2t[:, 0:128],
                                            macc[:, 1152:1280], OP.add)
                    nc.vector.tensor_tensor(msb[:, 896:1024], a2t[:, 128:256],
                                            macc[:, 1664:1792], OP.add)
                    TT = bpool.tile([128, 1024], dt.bfloat16, tag="TT")
                    for half in range(2):
                        tp = psH.tile([128, 512], dt.bfloat16, tag="psH")
                        for j in range(4):
                            cc = half * 512 + j * 128
                            nc.tensor.transpose(tp[:, j * 128:(j + 1) * 128],
                                                msb[:, cc:cc + 128], ident[:])
                        nc.any.tensor_copy(TT[:, half * 512:half * 512 + 512],
                                           tp[:])
                    # TT chunks: [A1|C0|C1|C2|B0|B1|B2|A2]
                    pm = psW.tile([128, 512], dt.float32, tag="psW")
                    for (a, c0, c1) in _RUNS[b]:
                        wslab = wtail[:, a * 512:(a + 1) * 512]
                        # out_s <- FS1^T A1 + FS2^T A2
                        nc.tensor.matmul(pm[:, c0:c1], wslab[:, 0:128],
                                         TT[:, c0:c1], start=True, stop=False,
                                         skip_group_check=True)
                        nc.tensor.matmul(pm[:, c0:c1], wslab[:, 128:256],
                                         TT[:, 896 + c0:896 + c1],
                                         start=False, stop=True,
                                         skip_group_check=True)
                        for i in range(3):
                            oc = 128 * (1 + i)
                            nc.tensor.matmul(
                                pm[:, oc + c0:oc + c1], wslab[:, 256:384],
                                TT[:, 512 + 128 * i + c0:512 + 128 * i + c1],
                                start=True, stop=False, skip_group_check=True)
                            nc.tensor.matmul(
                                pm[:, oc + c0:oc + c1], wslab[:, 384:512],
                                TT[:, 128 * (1 + i) + c0:128 * (1 + i) + c1],
                                start=False, stop=True, skip_group_check=True)
                    ob = bpool.tile([128, 512], dt.bfloat16, tag="ob")
                    nc.any.tensor_copy(ob[:], pm[:])
                    nc.sync.dma_start(
                        d_out[:, :, b * 128:(b + 1) * 128].rearrange(
                            "x p n -> p x n"),
                        ob[:].rearrange("p (x n) -> p x n", x=4))

    nc.compile()
    return nc


def kernel(**inputs):
    in_maps = _prep_host(**inputs)
    key = _LAYOUT["caps"]
    if key not in _CACHE:
        _CACHE[key] = _build_nc()
    nc = _CACHE[key]
    from concourse.bass_utils import run_bass_kernel_spmd
    res = run_bass_kernel_spmd(nc, in_maps, core_ids=list(range(NCORES)))
    return _assemble_output(res.results)
